# revision 12
# baseline (speedup 1.0000x reference)
"""Beam-search generator on 8 Trainium2 NeuronCores (Bass/Tile, SPMD).

Vocab-sharded strategy:
  - W [512, 32000] split by vocab across 8 cores (4000 each), SBUF-resident
    as bf16 hi/lo pair. Logits via 2 streamed passes per 500-col chunk with a
    packed stationary [xhi_kk | xlo_kk] (full 128-wide PE): PSUM rows 0:64
    accumulate hi*(whi+wlo), rows 64:128 lo*(whi+wlo); halves summed into an
    SBUF tile (ACT copy stages the lo rows since a DVE op may read only one
    PSUM input). 64 matmuls/step vs 96 for the 3-pass form, higher precision
    (includes lo*lo).
  - Each core: per-(row, chunk-half) top-8 (HW Max8/MaxIndex) + (max, sumexp)
    partials; one small AllGather per step; replicated merge on all cores.
  - Fin-set/batch-fin/record bookkeeping of step t is deferred into step t+1
    (executes on DVE during the E_tgt gather flight).
  - Throwaway PE matmuls keep the tensor-engine p-state warm across the
    AllGather and merge windows (FILL_AG/FILL_MG).
  - Sequences are reconstructed on host from per-step selection records.

Row order convention: row i = b + 16*k  (batch b, beam k).
Merge replica convention: partition p = b + 16*j (batch b, merge lane j).
"""
import sys

sys.path.insert(0, "/opt/trn_rl_repo/concourse")
sys.path.insert(0, "/opt/trn_rl_repo")

import numpy as np
import ml_dtypes

import concourse.bass as bass
import concourse.tile as tile
from concourse import mybir
from concourse.bass_types import AP
from concourse.bass_utils import run_bass_kernel_spmd
from bass_rust import ScopedClock

dt = mybir.dt
Alu = mybir.AluOpType
Act = mybir.ActivationFunctionType
Ax = mybir.AxisListType

B, S, D, V = 16, 128, 512, 32000
BEAM, L = 4, 16
NSTEP = L - 1
NC_ = 8
VC = V // NC_
NCH = 8
CW = VC // NCH
R = B * BEAM
FILL_AG = 56
FILL_MG = 14
NEG = np.float32(-1.0e9)
BOS, EOS = 1, 2

# --------- walrus sync-wait capacity workaround ---------
_uid = [0]


def _fresh_nop(engine, wait):
    _uid[0] += 1
    nop = mybir.InstNoOp(name=f"I-waitfix-{_uid[0]}", ins=[], outs=[])
    nop.engine = engine
    nop.sync_info = mybir.SyncInfo(on_wait=[wait], on_update=[])
    return nop


def _split_waits(nc):
    for fn in nc.m.functions:
        for bb in fn.blocks:
            insts = bb.instructions
            out = []
            changed = False
            for inst in insts:
                si = inst.sync_info
                waits = list(si.on_wait) if (si is not None and si.on_wait) else []
                if len(waits) > 1:
                    for w in waits[:-1]:
                        nop = _fresh_nop(inst.engine, w)
                        nc.inst_map[nop.name] = nop
                        out.append(nop)
                    si.on_wait = waits[-1:]
                    changed = True
                out.append(inst)
            if changed:
                del insts[:]
                insts.extend(out)


def _split_drain_and_barrier(self, tick_clock, wait_clock):
    nc = self.nc
    drain_inst = nc.sync.drain()
    wait_clock.add_sem_waits(
        drain_inst.ins, ScopedClock({None: tick_clock.global_clock})
    )
    si = drain_inst.ins.sync_info
    waits = list(si.on_wait) if (si is not None and si.on_wait) else []
    if len(waits) > 1:
        si.on_wait = waits[:1]
        for w in waits[1:]:
            extra = nc.sync.drain()
            esi = extra.ins.sync_info
            if esi is None:
                extra.ins.sync_info = mybir.SyncInfo(on_wait=[w], on_update=[])
            else:
                esi.on_wait = [w]
    nc.all_engine_barrier()
    popped = nc._tile_sem_poison_stack.pop()
    assert popped is self._sem_poison
    nc.clear_and_free_semaphores(list(self.sems.allocated().values()))
    nc.all_engine_barrier()


tile.TileContext._drain_and_barrier = _split_drain_and_barrier


def _ap(t_ap, off, free_dims, nparts=None):
    """AP view on a tile: keep its real partition pitch, custom free dims."""
    pitch = t_ap.ap[0][0]
    count = t_ap.ap[0][1] if nparts is None else nparts
    return AP(t_ap.tensor, t_ap.offset + off, [[pitch, count]] + list(free_dims))


_BUILD_CACHE = {}
_last_in_maps = None


def _build(with_bias):
    if with_bias in _BUILD_CACHE:
        return _BUILD_CACHE[with_bias]
    nc = bass.Bass("TRN2", target_bir_lowering=False, debug=False,
                   num_devices=NC_)
    f32, bf16, u32 = dt.float32, dt.bfloat16, dt.uint32

    whi_d = nc.dram_tensor("whi", [4, 128, VC], bf16, kind="ExternalInput").ap()
    wlo_d = nc.dram_tensor("wlo", [4, 128, VC], bf16, kind="ExternalInput").ap()
    etgt_d = nc.dram_tensor("etgt", [V, D], f32, kind="ExternalInput").ap()
    esrc_d = nc.dram_tensor("esrc", [V, D], f32, kind="ExternalInput").ap()
    srcf_d = nc.dram_tensor("srcf", [B, S], f32, kind="ExternalInput").ap()
    srcu_d = nc.dram_tensor("srcu", [S, B], u32, kind="ExternalInput").ap()
    vbase_d = nc.dram_tensor("vbase", [128, 1], f32, kind="ExternalInput").ap()
    rep16_64_d = nc.dram_tensor("rep16_64", [16, 64], f32, kind="ExternalInput").ap()
    rep16_128_d = nc.dram_tensor("rep16_128", [16, 128], f32, kind="ExternalInput").ap()
    rep16T_d = nc.dram_tensor("rep16T", [128, 16], f32, kind="ExternalInput").ap()
    sel8_d = nc.dram_tensor("sel8", [128, 8], f32, kind="ExternalInput").ap()
    sel4_d = nc.dram_tensor("sel4", [64, 4], f32, kind="ExternalInput").ap()
    i64_d = nc.dram_tensor("i64", [64, 64], f32, kind="ExternalInput").ap()
    i16_d = nc.dram_tensor("i16", [16, 16], f32, kind="ExternalInput").ap()
    ones1_d = nc.dram_tensor("ones1", [1, R], f32, kind="ExternalInput").ap()
    fold64_d = nc.dram_tensor("fold64", [128, 64], f32, kind="ExternalInput").ap()
    hm_d = nc.dram_tensor("hm", [128, 2], f32, kind="ExternalInput").ap()
    if with_bias:
        bsl_d = nc.dram_tensor("bsl", [1, VC], f32, kind="ExternalInput").ap()

    rec_d = nc.dram_tensor("rec", [NSTEP, B, 40], f32, kind="ExternalOutput").ap()

    agi_d = nc.dram_tensor("agi", [B, BEAM, 34], f32).ap()
    ago_d = nc.dram_tensor("ago", [NC_, B, BEAM, 34], f32,
                           addr_space="Shared").ap()

    from contextlib import ExitStack
    with tile.TileContext(nc) as tc, ExitStack() as ctx:
        sbP = ctx.enter_context(tc.tile_pool(name="persist", bufs=1))
        sbS = ctx.enter_context(tc.tile_pool(name="step", bufs=2))
        sbC = ctx.enter_context(tc.tile_pool(name="chunk", bufs=3))
        psL = ctx.enter_context(tc.tile_pool(name="psL", bufs=2, space="PSUM"))
        psT = ctx.enter_context(tc.tile_pool(name="psT", bufs=1, space="PSUM"))
        psM = ctx.enter_context(tc.tile_pool(name="psM", bufs=1, space="PSUM"))

        # ---------- constants ----------
        rep16_64 = sbP.tile([16, 64], f32, tag="rep16_64")
        nc.sync.dma_start(rep16_64[:], rep16_64_d[:])
        rep16_128 = sbP.tile([16, 128], f32, tag="rep16_128")
        nc.sync.dma_start(rep16_128[:], rep16_128_d[:])
        rep16T = sbP.tile([128, 16], f32, tag="rep16T")
        nc.sync.dma_start(rep16T[:], rep16T_d[:])
        sel8 = sbP.tile([128, 8], f32, tag="sel8")
        nc.sync.dma_start(sel8[:], sel8_d[:])
        sel4 = sbP.tile([64, 4], f32, tag="sel4")
        nc.sync.dma_start(sel4[:], sel4_d[:])
        i64 = sbP.tile([64, 64], f32, tag="i64")
        nc.sync.dma_start(i64[:], i64_d[:])
        i16 = sbP.tile([16, 16], f32, tag="i16")
        nc.sync.dma_start(i16[:], i16_d[:])
        vbase = sbP.tile([128, 1], f32, tag="vbase")
        nc.sync.dma_start(vbase[:], vbase_d[:])
        fold64 = sbP.tile([128, 64], f32, tag="fold64")
        nc.sync.dma_start(fold64[:], fold64_d[:])
        hm = sbP.tile([128, 2], f32, tag="hm")
        nc.sync.dma_start(hm[:], hm_d[:])
        ones1 = sbP.tile([1, R], f32, tag="ones1")
        nc.sync.dma_start(ones1[:], ones1_d[:])

        # ---------- weights ----------
        whi = sbP.tile([128, 4, VC], bf16, tag="whi")
        wlo = sbP.tile([128, 4, VC], bf16, tag="wlo")
        for kk in range(4):
            nc.sync.dma_start(whi[:, kk, :], whi_d[kk])
            nc.sync.dma_start(wlo[:, kk, :], wlo_d[kk])
        if with_bias:
            bsl = sbP.tile([1, VC], f32, tag="bsl")
            nc.sync.dma_start(bsl[:], bsl_d[:])
            brep = sbP.tile([R, VC], f32, tag="brep")
            for n in range(NCH):
                br_ps = psT.tile([R, CW], f32, tag="pst")
                nc.tensor.matmul(br_ps[:], lhsT=ones1[:],
                                 rhs=bsl[:, n * CW:(n + 1) * CW],
                                 start=True, stop=True)
                nc.vector.tensor_copy(brep[:, n * CW:(n + 1) * CW], br_ps[:])

        # ---------- encoder ----------
        if True:
            srcf = sbP.tile([B, S], f32, tag="srcf")
            nc.sync.dma_start(srcf[:], srcf_d[:])
            srcu = sbP.tile([S, B], u32, tag="srcu")
            nc.sync.dma_start(srcu[:], srcu_d[:])
            keep = sbP.tile([B, S], f32, tag="keep")
            nc.vector.tensor_scalar(keep[:], srcf[:], 0.0, None, op0=Alu.not_equal)
            cnt = sbP.tile([B, 1], f32, tag="cnt")
            nc.vector.tensor_reduce(cnt[:], keep[:], axis=Ax.X, op=Alu.add)
            cmax = sbP.tile([B, 1], f32, tag="cmax")
            nc.vector.tensor_scalar(cmax[:], cnt[:], 1.0, None, op0=Alu.max)
            cinv = sbP.tile([B, 1], f32, tag="cinv")
            nc.vector.reciprocal(cinv[:], cmax[:])

            # kscal = keep * (1/count) per batch row, then transpose
            kscal = sbP.tile([B, S], f32, tag="kscal")
            nc.vector.tensor_scalar(kscal[:], keep[:], cinv[:, :1], None,
                                    op0=Alu.mult)
            kscalT_ps = psT.tile([S, B], f32, tag="pst")
            nc.tensor.transpose(kscalT_ps[:], kscal[:], i16[:])
            kscalT = sbP.tile([S, B], f32, tag="kscalT")
            nc.vector.tensor_copy(kscalT[:], kscalT_ps[:])
            pooledT_ps = psM.tile([128, 4, B], f32, tag="mrgR")
            for b in range(B):
                ech = sbC.tile([S, D], f32, tag="ech")
                nc.gpsimd.indirect_dma_start(
                    out=ech[:], out_offset=None, in_=esrc_d[:],
                    in_offset=bass.IndirectOffsetOnAxis(ap=srcu[:, b:b + 1], axis=0),
                    bounds_check=V - 1, oob_is_err=False)
                for dc in range(4):
                    nc.tensor.matmul(pooledT_ps[:, dc, b:b + 1],
                                     lhsT=ech[:, 128 * dc:128 * (dc + 1)],
                                     rhs=kscalT[:, b:b + 1],
                                     start=True, stop=True)
            # poolXT[p, c, i=(b+16k)] = pooledT[p, c, b]
            poolXT = sbP.tile([128, 4, R], f32, tag="poolXT")
            for c in range(4):
                nc.vector.tensor_copy(
                    poolXT[:, c, :].rearrange("p (k b) -> p k b", k=4),
                    _ap(pooledT_ps[:], c * B, [[0, 4], [1, B]]))

        # ---------- decode state init ----------
        atokf = sbS.tile([B, BEAM], f32, tag="atokf")
        nc.vector.memset(atokf[:], float(BOS))
        alive_lp = sbS.tile([B, BEAM], f32, tag="alive_lp")
        nc.vector.memset(alive_lp[:], float(NEG))
        nc.vector.memset(alive_lp[:, 0:1], 0.0)
        cand12 = sbS.tile([B, 12], f32, tag="cand12")
        nc.vector.memset(cand12[:, 0:4], float(NEG))
        fl12 = sbS.tile([B, 12], f32, tag="fl12")
        nc.vector.memset(fl12[:, 0:4], 0.0)
        bfpen = sbS.tile([B, 1], f32, tag="bfpen")
        nc.vector.memset(bfpen[:], 0.0)
        batch_fin = sbS.tile([B, 1], f32, tag="batch_fin")
        nc.vector.memset(batch_fin[:], 0.0)

        # rolling deferred state (fin-set / batch_fin cluster of step t
        # is emitted early in step t+1, off the critical path)
        ST = {"cand12": cand12, "fl12": fl12, "batch_fin": batch_fin,
              "bfpen": bfpen, "deferred": None}

        for t in range(NSTEP):
            lpen = np.float32(t + 1)
            inv_lpen = np.float32(np.float32(1.0) / lpen)

            # ---- alive tokens -> [64, 1] u32 ----
            atr_ps = psT.tile([R, BEAM], f32, tag="pst")
            nc.tensor.matmul(atr_ps[:], lhsT=rep16_64[:], rhs=atokf[:],
                             start=True, stop=True)
            atm = sbS.tile([R, BEAM], f32, tag="atm")
            nc.vector.tensor_tensor(atm[:], atr_ps[:], sel4[:], op=Alu.mult)
            atok64 = sbS.tile([R, 1], f32, tag="atok64")
            nc.vector.tensor_reduce(atok64[:], atm[:], axis=Ax.X, op=Alu.max)
            idx64 = sbS.tile([R, 1], u32, tag="idx64")
            nc.vector.tensor_copy(idx64[:], atok64[:])

            # ---- gather x = E_tgt[tok] ----
            xg = sbS.tile([R, D], f32, tag="xg")
            nc.gpsimd.indirect_dma_start(
                out=xg[:], out_offset=None, in_=etgt_d[:],
                in_offset=bass.IndirectOffsetOnAxis(ap=idx64[:, :1], axis=0),
                bounds_check=V - 1, oob_is_err=False)

            # ---- deferred bookkeeping of step t-1 (runs during gather) ----
            if ST["deferred"] is not None:
                ST["deferred"]()
                ST["deferred"] = None

            # ---- x + pooled; transpose; bf16 hi/lo packed stationary ----
            xs_ps = psT.tile([128, 4, R], f32, tag="pst")
            for c in range(4):
                nc.tensor.transpose(xs_ps[:, c, :], xg[:, 128 * c:128 * (c + 1)],
                                    i64[:])
            xs = sbS.tile([128, 4 * R], f32, tag="xs")
            nc.vector.tensor_tensor(
                xs[:], xs_ps[:].rearrange("p c i -> p (c i)"),
                poolXT[:].rearrange("p c i -> p (c i)"), op=Alu.add)
            # xhl[:, kk, 0:64] = bf16(xs chunk kk); [:, kk, 64:128] = bf16(residual)
            xhl = sbS.tile([128, 4, 128], bf16, tag="xhl")
            nc.vector.tensor_copy(xhl[:, :, 0:64],
                                  xs[:].rearrange("p (k i) -> p k i", k=4))
            xlof = sbS.tile([128, 4 * R], f32, tag="xlof")
            nc.vector.tensor_tensor(xlof[:].rearrange("p (k i) -> p k i", k=4),
                                    xs[:].rearrange("p (k i) -> p k i", k=4),
                                    xhl[:, :, 0:64], op=Alu.subtract)
            nc.vector.tensor_copy(xhl[:, :, 64:128],
                                  xlof[:].rearrange("p (k i) -> p k i", k=4))

            # ---- logits: per 500-col chunk, 8 full-width matmuls
            # lhsT = [xhi_kk | xlo_kk] (M=128); psum rows 0:64 = hi terms,
            # rows 64:128 = lo terms; chunk pair summed into loch [128, CW]
            NP = NCH // 2
            cmh = sbS.tile([128, NP * 8], f32, tag="cmh")
            cgid = sbS.tile([128, NP * 8], f32, tag="cgid")
            s_all = sbS.tile([128, NP], f32, tag="s_all")
            for n in range(NP):
                lgs = []
                for h in range(2):
                    ch = 2 * n + h
                    lg = psL.tile([128, 512], f32, tag=f"lg{h}")
                    cs = slice(ch * CW, (ch + 1) * CW)
                    for kk in range(4):
                        nc.tensor.matmul(lg[:, 0:CW], lhsT=xhl[:, kk, :],
                                         rhs=whi[:, kk, cs],
                                         start=(kk == 0), stop=False)
                    for kk in range(4):
                        nc.tensor.matmul(lg[:, 0:CW], lhsT=xhl[:, kk, :],
                                         rhs=wlo[:, kk, cs],
                                         start=False, stop=(kk == 3))
                    lgs.append(lg)
                # HW: a DVE op may read only ONE input from PSUM, so stage
                # the lo-rows through SBUF on the (mostly idle) ACT engine
                locor = sbC.tile([128, CW], f32, tag="locor")
                nc.scalar.copy(locor[0:64, :], lgs[0][64:128, 0:CW])
                nc.scalar.copy(locor[64:128, :], lgs[1][64:128, 0:CW])
                loch = sbC.tile([128, CW], f32, tag="loch")
                nc.vector.tensor_tensor(loch[0:64, :], lgs[0][0:64, 0:CW],
                                        locor[0:64, :], op=Alu.add)
                nc.vector.tensor_tensor(loch[64:128, :], lgs[1][0:64, 0:CW],
                                        locor[64:128, :], op=Alu.add)
                if with_bias:
                    cs0 = slice((2 * n) * CW, (2 * n + 1) * CW)
                    cs1 = slice((2 * n + 1) * CW, (2 * n + 2) * CW)
                    nc.vector.tensor_tensor(loch[0:64, :], loch[0:64, :],
                                            brep[:, cs0], op=Alu.add)
                    nc.vector.tensor_tensor(loch[64:128, :], loch[64:128, :],
                                            brep[:, cs1], op=Alu.add)
                nc.vector.max(out=cmh[:, 8 * n:8 * (n + 1)], in_=loch[:])
                posn = sbC.tile([128, 8], u32, tag="posn")
                nc.vector.max_index(out=posn[:], in_max=cmh[:, 8 * n:8 * (n + 1)],
                                    in_values=loch[:])
                posf = sbC.tile([128, 8], f32, tag="posf")
                nc.vector.tensor_copy(posf[:], posn[:])
                # vbase here = core*VC + (p//64)*CW; chunk offset = 2n*CW
                nc.vector.tensor_scalar(cgid[:, 8 * n:8 * (n + 1)], posf[:],
                                        vbase[:, :1], float(2 * n * CW),
                                        op0=Alu.add, op1=Alu.add)
                negm = sbC.tile([128, 1], f32, tag="negm")
                nc.vector.tensor_scalar(negm[:], cmh[:, 8 * n:8 * n + 1], -1.0,
                                        None, op0=Alu.mult)
                exps = sbC.tile([128, CW], f32, tag="exps")
                nc.scalar.activation(exps[:], loch[:], Act.Exp, bias=negm[:, :1],
                                     scale=1.0, accum_out=s_all[:, n:n + 1])

            # ---- per-half-row top-8 + (M, S) ----
            v8h = sbS.tile([128, 8], f32, tag="v8h")
            nc.vector.max(out=v8h[:], in_=cmh[:])
            mk = sbS.tile([128, 8 * NP * 8], f32, tag="mk")
            nc.vector.tensor_tensor(
                mk[:].rearrange("p (i q) -> p i q", i=8),
                _ap(cmh[:], 0, [[0, 8], [1, NP * 8]]),
                _ap(v8h[:], 0, [[1, 8], [0, NP * 8]]), op=Alu.is_equal)
            mg = sbS.tile([128, 8 * NP * 8], f32, tag="mg")
            nc.vector.tensor_tensor(
                mg[:].rearrange("p (i q) -> p i q", i=8),
                mk[:].rearrange("p (i q) -> p i q", i=8),
                _ap(cgid[:], 0, [[0, 8], [1, NP * 8]]), op=Alu.mult)
            g8h = sbS.tile([128, 8], f32, tag="g8h")
            nc.vector.tensor_reduce(g8h[:],
                                    mg[:].rearrange("p (i q) -> p i q", i=8),
                                    axis=Ax.X, op=Alu.max)
            Mh = sbS.tile([128, 1], f32, tag="Mh")
            nc.vector.tensor_reduce(Mh[:], _ap(cmh[:], 0, [[8, NP]]),
                                    axis=Ax.X, op=Alu.max)
            dm = sbS.tile([128, NP], f32, tag="dm")
            nc.vector.tensor_scalar(dm[:], _ap(cmh[:], 0, [[8, NP]]), Mh[:, :1],
                                    None, op0=Alu.subtract)
            edm = sbS.tile([128, NP], f32, tag="edm")
            nc.scalar.activation(edm[:], dm[:], Act.Exp)
            sedm = sbS.tile([128, NP], f32, tag="sedm")
            nc.vector.tensor_tensor(sedm[:], s_all[:], edm[:], op=Alu.mult)
            Sh = sbS.tile([128, 1], f32, tag="Sh")
            nc.vector.tensor_reduce(Sh[:], sedm[:], axis=Ax.X, op=Alu.add)

            # ---- fold halves h=0/1 into per-row [64, 34] via 0/1 matmul ----
            rhs4 = sbS.tile([128, 36], f32, tag="rhs4")
            nc.vector.tensor_scalar(rhs4[:, 0:8], v8h[:], hm[:, 0:1], None,
                                    op0=Alu.mult)
            nc.vector.tensor_scalar(rhs4[:, 8:16], v8h[:], hm[:, 1:2], None,
                                    op0=Alu.mult)
            nc.vector.tensor_scalar(rhs4[:, 16:24], g8h[:], hm[:, 0:1], None,
                                    op0=Alu.mult)
            nc.vector.tensor_scalar(rhs4[:, 24:32], g8h[:], hm[:, 1:2], None,
                                    op0=Alu.mult)
            nc.vector.tensor_scalar(rhs4[:, 32:33], Mh[:], hm[:, 0:1], None,
                                    op0=Alu.mult)
            nc.vector.tensor_scalar(rhs4[:, 33:34], Mh[:], hm[:, 1:2], None,
                                    op0=Alu.mult)
            nc.vector.tensor_scalar(rhs4[:, 34:35], Sh[:], hm[:, 0:1], None,
                                    op0=Alu.mult)
            nc.vector.tensor_scalar(rhs4[:, 35:36], Sh[:], hm[:, 1:2], None,
                                    op0=Alu.mult)
            fold_ps = psT.tile([R, 36], f32, tag="pst")
            nc.tensor.matmul(fold_ps[:], lhsT=fold64[:], rhs=rhs4[:],
                             start=True, stop=True)
            agi = sbS.tile([R, 36], f32, tag="agi")
            nc.vector.tensor_copy(agi[:], fold_ps[:])
            # combine half (M, S): cols 32,33 = M0,M1; 34,35 = S0,S1
            Mrow = sbS.tile([R, 1], f32, tag="Mrow")
            nc.vector.tensor_tensor(Mrow[:], agi[:, 32:33], agi[:, 33:34],
                                    op=Alu.max)
            dm2 = sbS.tile([R, 2], f32, tag="dm2")
            nc.vector.tensor_scalar(dm2[:], agi[:, 32:34], Mrow[:, :1], None,
                                    op0=Alu.subtract)
            e2 = sbS.tile([R, 2], f32, tag="e2")
            nc.scalar.activation(e2[:], dm2[:], Act.Exp)
            s2 = sbS.tile([R, 2], f32, tag="s2")
            nc.vector.tensor_tensor(s2[:], e2[:], agi[:, 34:36], op=Alu.mult)
            Srow = sbS.tile([R, 1], f32, tag="Srow")
            nc.vector.tensor_reduce(Srow[:], s2[:], axis=Ax.X, op=Alu.add)
            nc.vector.tensor_copy(agi[:, 32:33], Mrow[:])
            nc.vector.tensor_copy(agi[:, 33:34], Srow[:])

            # ---- AllGather; alive_lp replication hoisted before it; PE kept
            # warm with throwaway matmuls while the collective is in flight
            nc.sync.dma_start(agi_d[:].rearrange("b k f -> k b f"), agi[:, 0:34])
            alr_ps = psT.tile([128, BEAM], f32, tag="pst")
            nc.tensor.matmul(alr_ps[:], lhsT=rep16_128[:], rhs=alive_lp[:],
                             start=True, stop=True)
            alrS = sbS.tile([128, BEAM], f32, tag="alrS")
            nc.vector.tensor_copy(alrS[:], alr_ps[:])
            nc.gpsimd.collective_compute(
                "AllGather", Alu.bypass, replica_groups=[list(range(NC_))],
                ins=[agi_d[:]], outs=[ago_d[:]])
            for _f in range(FILL_AG):
                flt = psL.tile([128, 512], f32, tag="lg0")
                nc.tensor.matmul(flt[:, 0:256], lhsT=xs[:, 0:128],
                                 rhs=xs[:], start=True, stop=True)
            mrg16 = sbS.tile([B, 1088], f32, tag="mrg16")
            src = AP(ago_d.tensor, 0, [[136, 16], [2176, 8], [1, 136]])
            nc.sync.dma_start(mrg16[:].rearrange("p (c q) -> p c q", c=8), src)
            mrgR = psM.tile([128, 1280], f32, tag="mrgR")
            nc.tensor.matmul(mrgR[:, 0:512], lhsT=rep16_128[:],
                             rhs=mrg16[:, 0:512], start=True, stop=True)
            nc.tensor.matmul(mrgR[:, 512:1024], lhsT=rep16_128[:],
                             rhs=_ap(mrg16[:], 512, [[1, 512]]),
                             start=True, stop=True)
            nc.tensor.matmul(mrgR[:, 1024:1088], lhsT=rep16_128[:],
                             rhs=_ap(mrg16[:], 1024, [[1, 64]]),
                             start=True, stop=True)
            for _f in range(FILL_MG):
                flt = psL.tile([128, 512], f32, tag="lg0")
                nc.tensor.matmul(flt[:, 0:256], lhsT=xs[:, 0:128],
                                 rhs=xs[:], start=True, stop=True)
            # free-dim layout within mrgR: (c:136, k:34, f:1); cands f=0:16,
            # gids f=16:32, M f=32, S f=33
            # ---- lse / adjusted scores ----
            M4 = sbS.tile([128, BEAM], f32, tag="M4")
            nc.vector.tensor_reduce(
                M4[:], _ap(mrgR[:], 32, [[34, 4], [136, 8]]), axis=Ax.X, op=Alu.max)
            dM = sbS.tile([128, BEAM * NC_], f32, tag="dM")
            nc.vector.tensor_tensor(
                dM[:].rearrange("p (k c) -> p k c", k=4),
                _ap(mrgR[:], 32, [[34, 4], [136, 8]]),
                _ap(M4[:], 0, [[1, 4], [0, 8]]), op=Alu.subtract)
            edM = sbS.tile([128, BEAM * NC_], f32, tag="edM")
            nc.scalar.activation(edM[:], dM[:], Act.Exp)
            sedM = sbS.tile([128, BEAM * NC_], f32, tag="sedM")
            nc.vector.tensor_tensor(
                sedM[:].rearrange("p (k c) -> p k c", k=4),
                edM[:].rearrange("p (k c) -> p k c", k=4),
                _ap(mrgR[:], 33, [[34, 4], [136, 8]]), op=Alu.mult)
            S4 = sbS.tile([128, BEAM], f32, tag="S4")
            nc.vector.tensor_reduce(S4[:],
                                    sedM[:].rearrange("p (k c) -> p k c", k=4),
                                    axis=Ax.X, op=Alu.add)
            lnS = sbS.tile([128, BEAM], f32, tag="lnS")
            nc.scalar.activation(lnS[:], S4[:], Act.Ln)
            # adj4 = (alive_lp - M) - lnS  == alive_lp - lse
            tmA = sbS.tile([128, BEAM], f32, tag="tmA")
            nc.vector.tensor_tensor(tmA[:], alrS[:], M4[:], op=Alu.subtract)
            adj4 = sbS.tile([128, BEAM], f32, tag="adj4")
            nc.vector.tensor_tensor(adj4[:], tmA[:], lnS[:], op=Alu.subtract)
            # fold the 1/lpen scale into the expand copy
            adjE = sbS.tile([128, 64], f32, tag="adjE")
            nc.vector.tensor_scalar(
                adjE[:].rearrange("p (k j) -> p k j", k=4),
                _ap(adj4[:], 0, [[1, 4], [0, 16]]),
                float(inv_lpen), None, op0=Alu.mult)
            scoreC = sbS.tile([128, 512], f32, tag="scoreC")
            nc.vector.scalar_tensor_tensor(
                out=scoreC[:].rearrange("p (c k j) -> p c k j", c=8, k=4),
                in0=_ap(mrgR[:], 0, [[136, 8], [34, 4], [1, 16]]),
                scalar=float(inv_lpen),
                in1=_ap(adjE[:], 0, [[0, 8], [16, 4], [1, 16]]),
                op0=Alu.mult, op1=Alu.add)

            # ---- merged top-8 (written straight into the record tile) ----
            rec = sbS.tile([B, 40], f32, tag="rec")
            vals8 = rec[:, 0:8]
            nc.vector.max(out=vals8, in_=scoreC[:16, :])

            # ---- token retrieval by value match ----
            vr_ps = psT.tile([128, 8], f32, tag="pst")
            nc.tensor.matmul(vr_ps[:], lhsT=rep16_128[:], rhs=vals8,
                             start=True, stop=True)
            vrm = sbS.tile([128, 8], f32, tag="vrm")
            nc.vector.tensor_tensor(vrm[:], vr_ps[:], sel8[:], op=Alu.mult)
            v_b = sbS.tile([128, 1], f32, tag="v_b")
            nc.vector.tensor_reduce(v_b[:], vrm[:], axis=Ax.X, op=Alu.max)
            mk2 = sbS.tile([128, 512], f32, tag="mk2")
            nc.vector.tensor_scalar(mk2[:], scoreC[:], v_b[:, :1], None,
                                    op0=Alu.is_equal)
            gm2 = sbS.tile([128, 512], f32, tag="gm2")
            nc.vector.tensor_tensor(
                gm2[:].rearrange("p (q j) -> p q j", q=32),
                mk2[:].rearrange("p (q j) -> p q j", q=32),
                _ap(mrgR[:], 16, [[34, 32], [1, 16]]), op=Alu.mult)
            gsel = sbS.tile([128, 1], f32, tag="gsel")
            nc.vector.tensor_reduce(gsel[:], gm2[:], axis=Ax.X, op=Alu.max)
            spread = sbS.tile([128, 8], f32, tag="spread")
            nc.vector.tensor_tensor(spread[:], _ap(gsel[:], 0, [[0, 8]]),
                                    sel8[:], op=Alu.mult)
            tok_ps = psT.tile([B, 8], f32, tag="pst")
            nc.tensor.matmul(tok_ps[:], lhsT=rep16T[:], rhs=spread[:],
                             start=True, stop=True)
            tok8 = rec[:, 8:16]
            nc.vector.tensor_copy(tok8, tok_ps[:])

            # ---- alive selection (critical part only; top-4 of curr8 are
            # always non-EOS so av8[:, 0:4] equals the alive scores) ----
            eos8 = sbS.tile([B, 8], f32, tag="eos8")
            nc.vector.tensor_scalar(eos8[:], tok8, float(EOS), None,
                                    op0=Alu.is_equal)
            curr8 = sbS.tile([B, 8], f32, tag="curr8")
            nc.vector.scalar_tensor_tensor(out=curr8[:], in0=eos8[:],
                                           scalar=float(NEG), in1=vals8,
                                           op0=Alu.mult, op1=Alu.add)
            av8 = sbS.tile([B, 8], f32, tag="av8")
            nc.vector.max(out=av8[:], in_=curr8[:])
            mka = sbS.tile([B, 4 * 8], f32, tag="mka")
            nc.vector.tensor_tensor(
                mka[:].rearrange("p (i j) -> p i j", i=4),
                _ap(curr8[:], 0, [[0, 4], [1, 8]]),
                _ap(av8[:], 0, [[1, 4], [0, 8]]), op=Alu.is_equal)
            atk_m = sbS.tile([B, 4 * 8], f32, tag="atk_m")
            nc.vector.tensor_tensor(
                atk_m[:].rearrange("p (i j) -> p i j", i=4),
                mka[:].rearrange("p (i j) -> p i j", i=4),
                _ap(tok8, 0, [[0, 4], [1, 8]]), op=Alu.mult)
            atokf = sbS.tile([B, BEAM], f32, tag="atokf")
            nc.vector.tensor_reduce(atokf[:],
                                    atk_m[:].rearrange("p (i j) -> p i j", i=4),
                                    axis=Ax.X, op=Alu.max)
            alive_lp = sbS.tile([B, BEAM], f32, tag="alive_lp")
            nc.vector.tensor_scalar(alive_lp[:], av8[:, 0:4], float(lpen),
                                    None, op0=Alu.mult)

            # ---- deferred cluster: records + fin set + batch_fin ----
            def _deferred(t=t, rec=rec, vals8=vals8, tok8=tok8, eos8=eos8,
                          curr8=curr8, av8=av8, scoreC=scoreC,
                          alive_lp=alive_lp, inv_lpen=inv_lpen):
                cand12, fl12 = ST["cand12"], ST["fl12"]
                pos8 = sbS.tile([B, 8], u32, tag="pos8")
                nc.vector.max_index(out=pos8[:], in_max=vals8,
                                    in_values=scoreC[:16, :])
                k8 = sbS.tile([B, 8], u32, tag="k8")
                nc.vector.tensor_scalar(k8[:], pos8[:], 4, 3,
                                        op0=Alu.logical_shift_right,
                                        op1=Alu.bitwise_and)
                nc.vector.tensor_copy(rec[:, 16:24], k8[:])
                aidx8 = sbS.tile([B, 8], u32, tag="aidx8")
                nc.vector.max_index(out=aidx8[:], in_max=av8[:],
                                    in_values=curr8[:])
                nc.vector.tensor_copy(rec[:, 24:32], aidx8[:])
                pen1 = sbS.tile([B, 8], f32, tag="pen1")
                nc.vector.tensor_scalar(pen1[:], eos8[:], 1.0, float(-NEG),
                                        op0=Alu.subtract, op1=Alu.mult)
                nc.vector.tensor_tensor(cand12[:, 4:12], vals8, pen1[:],
                                        op=Alu.add)
                nc.vector.tensor_scalar(cand12[:, 4:12], cand12[:, 4:12],
                                        ST["bfpen"][:, :1], None, op0=Alu.add)
                nc.vector.tensor_copy(fl12[:, 4:12], eos8[:])
                fv8 = sbS.tile([B, 8], f32, tag="fv8")
                nc.vector.max(out=fv8[:], in_=cand12[:])
                fidx8 = sbS.tile([B, 8], u32, tag="fidx8")
                nc.vector.max_index(out=fidx8[:], in_max=fv8[:],
                                    in_values=cand12[:])
                nc.vector.tensor_copy(rec[:, 32:40], fidx8[:])
                mkf = sbS.tile([B, 4 * 12], f32, tag="mkf")
                nc.vector.tensor_tensor(
                    mkf[:].rearrange("p (i j) -> p i j", i=4),
                    _ap(cand12[:], 0, [[0, 4], [1, 12]]),
                    _ap(fv8[:], 0, [[1, 4], [0, 12]]), op=Alu.is_equal)
                ffl_m = sbS.tile([B, 4 * 12], f32, tag="ffl_m")
                nc.vector.tensor_tensor(
                    ffl_m[:].rearrange("p (i j) -> p i j", i=4),
                    mkf[:].rearrange("p (i j) -> p i j", i=4),
                    _ap(fl12[:], 0, [[0, 4], [1, 12]]), op=Alu.mult)
                nfl = sbS.tile([B, BEAM], f32, tag="nfl")
                nc.vector.tensor_reduce(nfl[:],
                                        ffl_m[:].rearrange("p (i j) -> p i j",
                                                           i=4),
                                        axis=Ax.X, op=Alu.max)
                cand12_n = sbS.tile([B, 12], f32, tag="cand12")
                nc.vector.tensor_copy(cand12_n[:, 0:4], fv8[:, 0:4])
                fl12_n = sbS.tile([B, 12], f32, tag="fl12")
                nc.vector.tensor_copy(fl12_n[:, 0:4], nfl[:])
                # batch_fin
                sfm = sbS.tile([B, BEAM], f32, tag="sfm")
                nc.vector.tensor_tensor(sfm[:], fv8[:, 0:4], nfl[:],
                                        op=Alu.mult)
                lofin = sbS.tile([B, 1], f32, tag="lofin")
                nc.vector.tensor_reduce(lofin[:], sfm[:], axis=Ax.X, op=Alu.min)
                allfl = sbS.tile([B, 1], f32, tag="allfl")
                nc.vector.tensor_reduce(allfl[:], nfl[:], axis=Ax.X, op=Alu.min)
                pall = sbS.tile([B, 1], f32, tag="pall")
                nc.vector.tensor_scalar(pall[:], allfl[:], 1.0, float(-NEG),
                                        op0=Alu.subtract, op1=Alu.mult)
                lofin2 = sbS.tile([B, 1], f32, tag="lofin2")
                nc.vector.tensor_tensor(lofin2[:], lofin[:], pall[:],
                                        op=Alu.add)
                lb = sbS.tile([B, 1], f32, tag="lb")
                nc.vector.tensor_scalar(lb[:], alive_lp[:, 0:1],
                                        float(inv_lpen), None, op0=Alu.mult)
                ge = sbS.tile([B, 1], f32, tag="ge")
                nc.vector.tensor_tensor(ge[:], lofin2[:], lb[:], op=Alu.is_ge)
                batch_fin_n = sbS.tile([B, 1], f32, tag="batch_fin")
                nc.vector.tensor_tensor(batch_fin_n[:], ST["batch_fin"][:],
                                        ge[:], op=Alu.max)
                bfpen_n = sbS.tile([B, 1], f32, tag="bfpen")
                nc.vector.tensor_scalar(bfpen_n[:], batch_fin_n[:], float(NEG),
                                        None, op0=Alu.mult)
                ST["cand12"], ST["fl12"] = cand12_n, fl12_n
                ST["batch_fin"], ST["bfpen"] = batch_fin_n, bfpen_n
                nc.sync.dma_start(rec_d[t], rec[:])

            ST["deferred"] = _deferred

        # flush the last step's bookkeeping
        ST["deferred"]()
        ST["deferred"] = None

    _split_waits(nc)
    _BUILD_CACHE[with_bias] = nc
    return nc


# ======================================================================
# host side
# ======================================================================
def _host_constants():
    rep16_64 = np.zeros((16, 64), np.float32)
    for i in range(64):
        rep16_64[i % 16, i] = 1.0
    rep16_128 = np.zeros((16, 128), np.float32)
    for p in range(128):
        rep16_128[p % 16, p] = 1.0
    rep16T = np.ascontiguousarray(rep16_128.T)
    sel8 = np.zeros((128, 8), np.float32)
    for p in range(128):
        sel8[p, p // 16] = 1.0
    sel4 = np.zeros((64, 4), np.float32)
    for i in range(64):
        sel4[i, i // 16] = 1.0
    i64 = np.eye(64, dtype=np.float32)
    i16 = np.eye(16, dtype=np.float32)
    ones1 = np.ones((1, 64), np.float32)
    fold64 = np.zeros((128, 64), np.float32)
    for p in range(128):
        fold64[p, p % 64] = 1.0
    hm = np.zeros((128, 2), np.float32)
    hm[:64, 0] = 1.0
    hm[64:, 1] = 1.0
    return rep16_64, rep16_128, rep16T, sel8, sel4, i64, i16, ones1, fold64, hm


def _replay(rec):
    bidx = np.arange(B)[:, None]
    alive_seq = np.zeros((B, BEAM, L), np.int32)
    alive_seq[:, :, 0] = BOS
    fin_seq = np.zeros((B, BEAM, L), np.int32)
    for t in range(NSTEP):
        tok = np.rint(rec[t, :, 8:16]).astype(np.int32)
        k8 = np.rint(rec[t, :, 16:24]).astype(np.int64)
        aidx = np.rint(rec[t, :, 24:32]).astype(np.int64)[:, :4]
        fidx = np.rint(rec[t, :, 32:40]).astype(np.int64)[:, :4]
        topk_seq = alive_seq[bidx, k8].copy()
        topk_seq[:, :, t + 1] = tok
        alive_seq = topk_seq[bidx, aidx]
        cand_seq = np.concatenate([fin_seq, topk_seq], axis=1)
        fin_seq = cand_seq[bidx, fidx]
    return np.ascontiguousarray(fin_seq[:, 0].astype(np.int32))


def build_in_maps(src_input, E_src, E_tgt, W, b, with_bias):
    W_hi = W.astype(ml_dtypes.bfloat16)
    W_lo = (W - W_hi.astype(np.float32)).astype(ml_dtypes.bfloat16)
    (rep16_64, rep16_128, rep16T, sel8, sel4, i64, i16,
     ones1, fold64, hm) = _host_constants()
    srcf = src_input.astype(np.float32)
    srcu = np.ascontiguousarray(src_input.astype(np.uint32).T)

    in_maps = []
    for c in range(NC_):
        vs = slice(c * VC, (c + 1) * VC)
        m = {
            "whi": np.ascontiguousarray(W_hi[:, vs]).reshape(4, 128, VC),
            "wlo": np.ascontiguousarray(W_lo[:, vs]).reshape(4, 128, VC),
            "etgt": E_tgt, "esrc": E_src,
            "srcf": srcf, "srcu": srcu,
            "vbase": (np.float32(c * VC)
                      + (np.arange(128) // 64).astype(np.float32)[:, None]
                      * np.float32(CW)).astype(np.float32),
            "rep16_64": rep16_64, "rep16_128": rep16_128, "rep16T": rep16T,
            "sel8": sel8, "sel4": sel4, "i64": i64, "i16": i16,
            "ones1": ones1, "fold64": fold64, "hm": hm,
        }
        if with_bias:
            m["bsl"] = np.ascontiguousarray(b[vs])[None, :]
        in_maps.append(m)
    return in_maps


def kernel(src_input, E_src, E_tgt, W, b):
    src_input = np.asarray(src_input)
    E_src = np.ascontiguousarray(np.asarray(E_src, dtype=np.float32))
    E_tgt = np.ascontiguousarray(np.asarray(E_tgt, dtype=np.float32))
    W = np.asarray(W, dtype=np.float32)
    b = np.asarray(b, dtype=np.float32)

    with_bias = bool(np.any(b))
    nc = _build(with_bias)
    in_maps = build_in_maps(src_input, E_src, E_tgt, W, b, with_bias)

    global _last_in_maps
    _last_in_maps = in_maps
    res = run_bass_kernel_spmd(nc, in_maps, list(range(NC_)))
    rec = res.results[0]["rec"]
    return _replay(rec)


if __name__ == "__main__":
    pass

